# revision 1
# baseline (speedup 1.0000x reference)
"""CrossAttention (RoPE, 16 heads, C=1024) Trainium2 Bass kernel.

Sharding: DP over batch (4) x TP over heads (2 groups of 8) = 8 cores.
Each core computes, for its (batch b, head-group g):
  Q/K/V projections (column-parallel), RoPE, scores^T, exp (softmax without
  max-subtraction; logits are bounded), PV with an appended ones-column for
  the row-sums, late normalization, and the row-parallel output projection
  producing a partial out^T.  The host sums the two head-group partials.

All matmuls run as float32r (reduced-mantissa fp32) at full PE rate.

Pipeline: A (Q proj+RoPE) -> B (K proj+RoPE) -> pass0 (V proj fused with
attention for heads 0-2, key-chunk-outer, PE-bound) -> pass1 (attention for
heads 3-7, head-outer, exp-bound) -> E (output projection).

Layout notes (per core):
  qT  [C, Nq]   kT [C, Nk]   vT [C, Nk]      (activations, transposed on host)
  wqT/wkT/wvT [C, 512]   wpT [CH, C]          (weight slices, host-prepped)
  q/k rope tables [128, N] (64-row table duplicated; rows r use inv_freq[r%32])
  rT [128, 128]  block-diag rotate-half matrix:  rot(x) = rT.T @ x
  RoPE identity used:  rope(x) = x*cos + rT.T @ (x*sin)   (sin is 32-periodic
  along d, and rotate-half is a signed permutation within mod-32 classes).
"""

import sys

if "/opt/trn_rl_repo" not in sys.path:
    sys.path.insert(0, "/opt/trn_rl_repo")

import numpy as np
from contextlib import ExitStack

import concourse.bass as bass
import concourse.tile as tile
from concourse import bacc, mybir

F32 = mybir.dt.float32
F32R = mybir.dt.float32r
EXP = mybir.ActivationFunctionType.Exp

# problem constants
B, Nq, Nk, C = 4, 512, 2048, 1024
H, D = 16, 64
HL = 8            # heads per core
CH = HL * D       # 512 local channels
NPAIR = HL // 2   # 4 pair-chunks of 128 output dims
SC = Nk // 128    # 16 key chunks of 128
SB = Nk // 512    # 4 key blocks of 512
ROPE_BASE = 10000.0
SCALE = float(D) ** -0.5

P0H = 3                      # heads fused with the V projection in pass0
DG = [3, 3, 3, 3, 3, 1]      # pass1 exp grouping (s-chunks per PSUM tile)


def _ld3(nc, dst, src_2d, width=512):
    """One DMA loading a [N*128, width] DRAM region into a [128, N*width]
    tile (row-chunk ci lands at columns [ci*width, (ci+1)*width))."""
    nc.sync.dma_start(
        dst[:].rearrange("p (a s) -> p a s", s=width),
        src_2d.rearrange("(a p) s -> p a s", p=128))


def build_nc(iters: int = 1):
    nc = bacc.Bacc("TRN2", target_bir_lowering=False, debug=False)

    qT = nc.dram_tensor("qT", [C, Nq], F32, kind="ExternalInput")
    kT = nc.dram_tensor("kT", [C, Nk], F32, kind="ExternalInput")
    vT = nc.dram_tensor("vT", [C, Nk], F32, kind="ExternalInput")
    wqT = nc.dram_tensor("wqT", [C, CH], F32, kind="ExternalInput")
    wkT = nc.dram_tensor("wkT", [C, CH], F32, kind="ExternalInput")
    wvT = nc.dram_tensor("wvT", [C, CH], F32, kind="ExternalInput")
    wpT = nc.dram_tensor("wpT", [CH, C], F32, kind="ExternalInput")
    bpT = nc.dram_tensor("bpT", [128, 8], F32, kind="ExternalInput")
    qcos = nc.dram_tensor("qcos", [128, Nq], F32, kind="ExternalInput")
    qsin = nc.dram_tensor("qsin", [128, Nq], F32, kind="ExternalInput")
    kcos = nc.dram_tensor("kcos", [128, Nk], F32, kind="ExternalInput")
    ksin = nc.dram_tensor("ksin", [128, Nk], F32, kind="ExternalInput")
    rT = nc.dram_tensor("rT", [128, 128], F32, kind="ExternalInput")
    outT = nc.dram_tensor("outT", [C, Nq], F32, kind="ExternalOutput")

    def head_slices(h):
        """kr/qr pair index and row offset for local head h."""
        return h // 2, 64 * (h % 2)

    with tile.TileContext(nc) as tc, ExitStack() as top:
        const = top.enter_context(tc.tile_pool(name="const", bufs=1))
        rt_t = const.tile([128, 128], F32R, tag="rt", name="rt")
        nc.sync.dma_start(rt_t[:], rT[:].bitcast(F32R))
        bp_t = const.tile([128, 8], F32, tag="bp", name="bp")
        nc.sync.dma_start(bp_t[:], bpT[:])
        ones_f32 = const.tile([128, 128], F32, tag="ones_f32", name="ones_f32")
        nc.vector.memset(ones_f32[:], 1.0)
        ones_t = const.tile([128, 64], F32R, tag="ones", name="ones")
        nc.vector.tensor_copy(ones_t[:], ones_f32[:, 0:64])

        for _ in range(iters):
            with ExitStack() as it_stack:
                qkr = it_stack.enter_context(tc.tile_pool(name="qkr", bufs=1))
                qr_t = [qkr.tile([128, Nq], F32R, tag=f"qr{m}", name=f"qr{m}")
                        for m in range(NPAIR)]
                kr_t = [qkr.tile([128, Nk], F32R, tag=f"kr{m}", name=f"kr{m}")
                        for m in range(NPAIR)]
                vxt = it_stack.enter_context(tc.tile_pool(name="vxt", bufs=1))
                v65 = vxt.tile([128, SC * 520], F32R, tag="v65", name="v65")
                xt_t = [vxt.tile([65, Nq], F32R, tag=f"xt{h}", name=f"xt{h}")
                        for h in range(HL)]
                # attention-prob tiles + normalization scratch span pass0+pass1
                dpool = it_stack.enter_context(tc.tile_pool(name="dpool",
                                                            bufs=1))

                def exp_tile(width):
                    return dpool.tile([128, width], F32R, tag="pt", name="pt",
                                      bufs=2, padded_shape=[128, 1536])

                def normalize(h):
                    """x = x / rowsum, in place on xt rows 0:64 (pr from the
                    caller's PSUM pool via matmul against the ones row)."""
                    inv = dpool.tile([64, Nq], F32, tag="inv", name="inv",
                                     bufs=2)
                    return inv

                # ========== Phases A/B: Q/K projections + RoPE ==========
                with ExitStack() as ab:
                    stream = ab.enter_context(
                        tc.tile_pool(name="stream", bufs=2))
                    wstage = ab.enter_context(
                        tc.tile_pool(name="wstage", bufs=2))

                    with ExitStack() as abx:
                        ppsum = abx.enter_context(
                            tc.tile_pool(name="ppsum", bufs=2, space="PSUM"))
                        rpsum = abx.enter_context(
                            tc.tile_pool(name="rpsum", bufs=2, space="PSUM"))
                        ktbl = abx.enter_context(
                            tc.tile_pool(name="ktbl", bufs=2))
                        rope = abx.enter_context(
                            tc.tile_pool(name="rope", bufs=1))
                        evac = abx.enter_context(
                            tc.tile_pool(name="evac", bufs=2))

                        def rope_block(x_psum, cos_ap, sin_ap, out_ap, width):
                            """out = x*cos + rT.T @ (x*sin); ACT evacuates."""
                            xs = evac.tile([128, width], F32, tag="xs",
                                           name="xs")
                            nc.scalar.copy(xs[:], x_psum[:])
                            tsin = rope.tile([128, width], F32R, tag="tsin",
                                             name="tsin")
                            nc.vector.tensor_mul(tsin[:], xs[:], sin_ap)
                            prot = rpsum.tile([128, width], F32, tag="prot",
                                              name="prot")
                            nc.tensor.matmul(prot[:], rt_t[:], tsin[:],
                                             start=True, stop=True)
                            tcos = rope.tile([128, width], F32, tag="tcos",
                                             name="tcos")
                            nc.vector.tensor_mul(tcos[:], xs[:], cos_ap)
                            nc.vector.tensor_add(out_ap, tcos[:], prot[:])

                        # ---- A: Q projection + RoPE ----
                        qc_t = ktbl.tile([128, Nq], F32, tag="kcos",
                                         name="qcos")
                        nc.sync.dma_start(qc_t[:], qcos[:])
                        qs_t = ktbl.tile([128, Nq], F32, tag="ksin",
                                         name="qsin")
                        nc.sync.dma_start(qs_t[:], qsin[:])
                        wq_t = wstage.tile([128, 8 * CH], F32R, tag="w",
                                           name="wq")
                        _ld3(nc, wq_t, wqT[:].bitcast(F32R), CH)
                        qt_t = stream.tile([128, 4096], F32R, tag="s",
                                           name="qt")
                        _ld3(nc, qt_t, qT[:].bitcast(F32R))

                        for m in range(NPAIR):
                            pq = ppsum.tile([128, Nq], F32, tag="pq",
                                            name="pq")
                            for ci in range(8):
                                nc.tensor.matmul(
                                    pq[:],
                                    wq_t[:, ci * CH + m * 128:
                                         ci * CH + (m + 1) * 128],
                                    qt_t[:, ci * 512:(ci + 1) * 512],
                                    start=(ci == 0), stop=(ci == 7))
                            rope_block(pq, qc_t[:], qs_t[:], qr_t[m][:], Nq)

                        # ---- B: K projection + RoPE ----
                        wk_t = wstage.tile([128, 8 * CH], F32R, tag="w",
                                           name="wk")
                        _ld3(nc, wk_t, wkT[:].bitcast(F32R), CH)
                        for sbi in range(SB):
                            sl = slice(sbi * 512, (sbi + 1) * 512)
                            kc_t = ktbl.tile([128, 512], F32, tag="kcos",
                                             name="kcos")
                            nc.sync.dma_start(kc_t[:], kcos[:, sl])
                            ks_t = ktbl.tile([128, 512], F32, tag="ksin",
                                             name="ksin")
                            nc.sync.dma_start(ks_t[:], ksin[:, sl])
                            kt_t = stream.tile([128, 4096], F32R, tag="s",
                                               name="kt")
                            _ld3(nc, kt_t, kT[:, sl].bitcast(F32R))
                            for m in range(NPAIR):
                                pk = ppsum.tile([128, 512], F32, tag="pq",
                                                name="pk")
                                for ci in range(8):
                                    nc.tensor.matmul(
                                        pk[:],
                                        wk_t[:, ci * CH + m * 128:
                                             ci * CH + (m + 1) * 128],
                                        kt_t[:, ci * 512:(ci + 1) * 512],
                                        start=(ci == 0), stop=(ci == 7))
                                rope_block(pk, kc_t[:], ks_t[:],
                                           kr_t[m][:, sl], 512)

                    # ===== pass0: V projection + attention heads 0..P0H-1,
                    # key-chunk-outer (wstage/stream stay open for wv/vT) ====
                    ones_cols = v65[:].rearrange(
                        "p (n w) -> p n w", w=65)[:, :, 64:65]
                    nc.vector.tensor_copy(
                        ones_cols,
                        ones_f32[:, 0:SC * 8].rearrange(
                            "p (n w) -> p n w", w=1))
                    with ExitStack() as ph:
                        pv_pool = ph.enter_context(
                            tc.tile_pool(name="pv0", bufs=2, space="PSUM"))
                        psc_pool = ph.enter_context(
                            tc.tile_pool(name="psc0", bufs=1, space="PSUM"))
                        pxt_pool = ph.enter_context(
                            tc.tile_pool(name="pxt0", bufs=1, space="PSUM"))

                        wv_t = wstage.tile([128, 8 * CH], F32R, tag="w",
                                           name="wv")
                        _ld3(nc, wv_t, wvT[:].bitcast(F32R), CH)
                        pxt0 = [pxt_pool.tile([65, Nq], F32, tag=f"px{h}",
                                              name=f"px{h}")
                                for h in range(P0H)]
                        for sbi in range(SB):
                            sl = slice(sbi * 512, (sbi + 1) * 512)
                            vt_t = stream.tile([128, 4096], F32R, tag="s",
                                               name="vt")
                            _ld3(nc, vt_t, vT[:, sl].bitcast(F32R))
                            for scj in range(4):
                                sc = sbi * 4 + scj
                                pv = pv_pool.tile([128, CH], F32, tag="pv",
                                                  name="pv")
                                for ci in range(8):
                                    nc.tensor.matmul(
                                        pv[:],
                                        vt_t[:, ci * 512 + scj * 128:
                                             ci * 512 + (scj + 1) * 128],
                                        wv_t[:, ci * CH:(ci + 1) * CH],
                                        start=(ci == 0), stop=(ci == 7))
                                dst = v65[:, sc * 520:(sc + 1) * 520
                                          ].rearrange(
                                              "p (n w) -> p n w",
                                              w=65)[:, :, 0:64]
                                nc.scalar.copy(
                                    dst,
                                    pv[:].rearrange("p (n w) -> p n w", w=64))

                                psc = psc_pool.tile([128, 512 * P0H], F32,
                                                    tag="psc", name="psc")
                                for hj in range(P0H):
                                    p, r0 = head_slices(hj)
                                    nc.tensor.matmul(
                                        psc[:, hj * 512:(hj + 1) * 512],
                                        kr_t[p][r0:r0 + 64,
                                                sc * 128:(sc + 1) * 128],
                                        qr_t[p][r0:r0 + 64, :],
                                        start=True, stop=True)
                                pt = exp_tile(512 * P0H)
                                nc.scalar.activation(pt[:], psc[:], EXP,
                                                     scale=SCALE)
                                for hj in range(P0H):
                                    nc.tensor.matmul(
                                        pxt0[hj][:],
                                        v65[:, sc * 520 + hj * 65:
                                            sc * 520 + hj * 65 + 65],
                                        pt[:, hj * 512:(hj + 1) * 512],
                                        start=(sc == 0), stop=(sc == SC - 1),
                                        skip_group_check=True)
                        for hj in range(P0H):
                            nc.vector.tensor_copy(xt_t[hj][:], pxt0[hj][:])

                # ===== pass1: attention heads P0H..7, head-outer =====
                with ExitStack() as phd:
                    wp_pool = phd.enter_context(tc.tile_pool(name="wpp",
                                                             bufs=1))
                    wp_t = wp_pool.tile([64, HL * C], F32R, tag="wp",
                                        name="wp")
                    nc.sync.dma_start(
                        wp_t[:].rearrange("p (a s) -> p a s", s=C),
                        wpT[:].bitcast(F32R).rearrange("(a p) s -> p a s",
                                                       p=64))

                    with ExitStack() as ph:
                        psc_pool = ph.enter_context(
                            tc.tile_pool(name="psc1", bufs=2, space="PSUM"))
                        pxt_pool = ph.enter_context(
                            tc.tile_pool(name="pxt1", bufs=2, space="PSUM"))

                        def do_normalize(h):
                            pr = pxt_pool.tile([64, Nq], F32, tag="pxt",
                                               name="pr")
                            nc.tensor.matmul(pr[:], ones_t[64:65, :],
                                             xt_t[h][64:65, :],
                                             start=True, stop=True)
                            inv = normalize(h)
                            nc.vector.reciprocal(inv[:], pr[:])
                            nc.vector.tensor_mul(
                                xt_t[h][0:64, :],
                                xt_t[h][0:64, :].bitcast(F32), inv[:])

                        for hj in range(P0H):
                            do_normalize(hj)

                        for h in range(P0H, HL):
                            p, r0 = head_slices(h)
                            pxt = pxt_pool.tile([65, Nq], F32, tag="pxt",
                                                name="pxt")
                            sc0 = 0
                            for gw in DG:
                                psc = psc_pool.tile([128, 512 * gw], F32,
                                                    tag="psc", name="psc")
                                for j in range(gw):
                                    sc = sc0 + j
                                    nc.tensor.matmul(
                                        psc[:, j * 512:(j + 1) * 512],
                                        kr_t[p][r0:r0 + 64,
                                                sc * 128:(sc + 1) * 128],
                                        qr_t[p][r0:r0 + 64, :],
                                        start=True, stop=True)
                                pt = exp_tile(512 * gw)
                                nc.scalar.activation(pt[:], psc[:], EXP,
                                                     scale=SCALE)
                                for j in range(gw):
                                    sc = sc0 + j
                                    nc.tensor.matmul(
                                        pxt[:],
                                        v65[:, sc * 520 + h * 65:
                                            sc * 520 + h * 65 + 65],
                                        pt[:, j * 512:(j + 1) * 512],
                                        start=(sc == 0), stop=(sc == SC - 1),
                                        skip_group_check=True)
                                sc0 += gw
                            nc.vector.tensor_copy(xt_t[h][:], pxt[:])
                            do_normalize(h)

                    # ========== E: output projection ==========
                    with ExitStack() as ph:
                        pool = ph.enter_context(tc.tile_pool(name="phE",
                                                             bufs=3))
                        po_pool = ph.enter_context(
                            tc.tile_pool(name="poE", bufs=4, space="PSUM"))

                        for j in range(8):
                            po = po_pool.tile([128, Nq], F32, tag="po",
                                              name="po")
                            for h in range(HL):
                                nc.tensor.matmul(
                                    po[:],
                                    wp_t[:, h * C + j * 128:
                                         h * C + (j + 1) * 128],
                                    xt_t[h][0:64, :], start=(h == 0),
                                    stop=(h == 7))
                            osb = pool.tile([128, Nq], F32, tag="osb",
                                            name="osb")
                            nc.vector.tensor_scalar_add(osb[:], po[:],
                                                        bp_t[:, j:j + 1])
                            nc.sync.dma_start(
                                outT[j * 128:(j + 1) * 128, :], osb[:])

    nc.compile()
    return nc


def prep_inputs(query, key, value, qpos, kpos, Wq, Wk, Wv, Wp, bp):
    """Build per-core input maps (8 cores: core = 2*b + g)."""
    invf = (1.0 / ROPE_BASE ** (np.arange(0, D, 2, dtype=np.float32) / D)
            ).astype(np.float32)
    rows64 = invf[np.arange(64) % 32]          # [64]

    R64 = np.zeros((64, 64), dtype=np.float32)
    for r in range(32):
        R64[r, r + 32] = -1.0
        R64[r + 32, r] = 1.0
    rT128 = np.zeros((128, 128), dtype=np.float32)
    rT128[0:64, 0:64] = R64.T
    rT128[64:128, 64:128] = R64.T

    in_maps = []
    for core in range(8):
        b, g = core // 2, core % 2
        cols = slice(g * CH, (g + 1) * CH)
        qang = rows64[:, None] * np.asarray(qpos[b], np.float32)[None, :]
        kang = rows64[:, None] * np.asarray(kpos[b], np.float32)[None, :]
        m = {
            "qT": np.ascontiguousarray(np.asarray(query[b], np.float32).T),
            "kT": np.ascontiguousarray(np.asarray(key[b], np.float32).T),
            "vT": np.ascontiguousarray(np.asarray(value[b], np.float32).T),
            "wqT": np.ascontiguousarray(np.asarray(Wq, np.float32)[cols, :].T),
            "wkT": np.ascontiguousarray(np.asarray(Wk, np.float32)[cols, :].T),
            "wvT": np.ascontiguousarray(np.asarray(Wv, np.float32)[cols, :].T),
            "wpT": np.ascontiguousarray(np.asarray(Wp, np.float32)[:, cols].T),
            "bpT": (np.ascontiguousarray(
                        np.asarray(bp, np.float32).reshape(8, 128).T)
                    if g == 0 else np.zeros((128, 8), np.float32)),
            "qcos": np.ascontiguousarray(
                np.tile(np.cos(qang), (2, 1)).astype(np.float32)),
            "qsin": np.ascontiguousarray(
                np.tile(np.sin(qang), (2, 1)).astype(np.float32)),
            "kcos": np.ascontiguousarray(
                np.tile(np.cos(kang), (2, 1)).astype(np.float32)),
            "ksin": np.ascontiguousarray(
                np.tile(np.sin(kang), (2, 1)).astype(np.float32)),
            "rT": rT128,
        }
        in_maps.append(m)
    return in_maps


_NC_CACHE = {}


def _get_nc(iters=1):
    if iters not in _NC_CACHE:
        _NC_CACHE[iters] = build_nc(iters)
    return _NC_CACHE[iters]


def kernel(query, key, value, qpos, kpos, Wq, Wk, Wv, Wp, bp):
    from concourse.bass_utils import run_bass_kernel_spmd

    nc = _get_nc()
    in_maps = prep_inputs(query, key, value, qpos, kpos, Wq, Wk, Wv, Wp, bp)
    res = run_bass_kernel_spmd(nc, in_maps, list(range(8)))
    out = np.zeros((B, Nq, C), dtype=np.float32)
    for core in range(8):
        out[core // 2] += res.results[core]["outT"].T
    return out



# revision 11
# speedup vs baseline: 1.2508x; 1.2508x over previous
"""CrossAttention (RoPE, 16 heads, C=1024) Trainium2 Bass kernel.

Sharding: DP over batch (4) x TP over heads (2 groups of 8) = 8 cores.
Each core computes, for its (batch b, head-group g):
  Q/K/V projections (column-parallel), RoPE, scores, exp (softmax without
  max-subtraction; logits are bounded), transposed PV accumulation with an
  appended ones-column for the row-sums, late normalization, and the
  row-parallel output projection producing a partial out^T.  The host sums
  the two head-group partials.

All data is bf16 in SBUF (f32 PSUM accumulation), halving DMA traffic and
enabling small-moving-dim matmuls at full rate.

Pipeline (engine balance):
  A: Q proj + RoPE.
  B: K proj + RoPE, with scores+exp for heads 0..3 interleaved per key
     block so the ACT engine starts the softmax early (probs are saved
     in SBUF until pass0b).
  pass0a: V projection streamed per 128-key chunk into v65; heads 4..6
     run scores+exp here into rotating prob buffers.
  pass0b: transposed PV (out [q, 64ch+1ones], moving dim 65) — one PSUM
     accumulation group per (head, q-chunk), one bank per group (PSUM
     start_tensor_calc zeroes a whole 2KB bank, so concurrent groups must
     not share banks).  Head 7's scores+exp overlap this phase.
     Normalization happens per group: DVE reciprocal of the ones column,
     Pool tensor_scalar multiply into xq.
  tail: PE transposes xq back to [ch, q], paired output projection
     (contraction 128 = head pair), bias add, DMA out.

Layout notes (per core):
  qT  [C, Nq]   kT [C, Nk]   vT [C, Nk]      (activations, transposed, bf16)
  wqT/wkT/wvT [C, 512]   wpT [128, 4*C]      (weight slices, host-prepped)
  v65 [128, SC*520]: per key-chunk sc, per head h a [128, 65] block of
  64 V-channels plus a ones column.
  rope tables [128, N] bf16 (64-row table duplicated; rows use
  inv_freq[r%32]);  rT [128, 128] block-diag rotate-half matrix.
  RoPE identity:  rope(x) = x*cos + rT.T @ (x*sin).
"""

import sys

if "/opt/trn_rl_repo" not in sys.path:
    sys.path.insert(0, "/opt/trn_rl_repo")

import numpy as np
import ml_dtypes
from contextlib import ExitStack

import concourse.bass as bass
import concourse.tile as tile
from concourse import bacc, mybir

F32 = mybir.dt.float32
B16 = mybir.dt.bfloat16
EXP = mybir.ActivationFunctionType.Exp

# problem constants
B, Nq, Nk, C = 4, 512, 2048, 1024
H, D = 16, 64
HL = 8            # heads per core
CH = HL * D       # 512 local channels
NPAIR = HL // 2   # 4 pair-chunks of 128 channels
SC = Nk // 128    # 16 key chunks of 128
SB = Nk // 512    # 4 key blocks of 512
ROPE_BASE = 10000.0
SCALE = float(D) ** -0.5

NB_EARLY = 4      # heads whose scores+exp run during phase B
NB_MID = 3        # heads whose scores+exp run during pass0a (rotating bufs)


def _ld3(nc, dst, src_2d, width=512):
    """One DMA loading a [N*128, width] DRAM region into a [128, N*width]
    tile (row-chunk ci lands at columns [ci*width, (ci+1)*width))."""
    nc.sync.dma_start(
        dst[:].rearrange("p (a s) -> p a s", s=width),
        src_2d.rearrange("(a p) s -> p a s", p=128))


def build_nc(iters: int = 1):
    nc = bacc.Bacc("TRN2", target_bir_lowering=False, debug=False)

    qT = nc.dram_tensor("qT", [C, Nq], B16, kind="ExternalInput")
    kT = nc.dram_tensor("kT", [C, Nk], B16, kind="ExternalInput")
    vT = nc.dram_tensor("vT", [C, Nk], B16, kind="ExternalInput")
    wqT = nc.dram_tensor("wqT", [C, CH], B16, kind="ExternalInput")
    wkT = nc.dram_tensor("wkT", [C, CH], B16, kind="ExternalInput")
    wvT = nc.dram_tensor("wvT", [C, CH], B16, kind="ExternalInput")
    wpT = nc.dram_tensor("wpT", [128, NPAIR * C], B16, kind="ExternalInput")
    bpT = nc.dram_tensor("bpT", [128, 8], F32, kind="ExternalInput")
    qcos = nc.dram_tensor("qcos", [128, Nq], B16, kind="ExternalInput")
    qsin = nc.dram_tensor("qsin", [128, Nq], B16, kind="ExternalInput")
    kcos = nc.dram_tensor("kcos", [128, Nk], B16, kind="ExternalInput")
    ksin = nc.dram_tensor("ksin", [128, Nk], B16, kind="ExternalInput")
    rT = nc.dram_tensor("rT", [128, 128], B16, kind="ExternalInput")
    idT = nc.dram_tensor("idT", [128, 128], B16, kind="ExternalInput")
    outT = nc.dram_tensor("outT", [C, Nq], F32, kind="ExternalOutput")

    def head_slices(h):
        """kr/qr pair index and row offset for local head h."""
        return h // 2, 64 * (h % 2)

    with tile.TileContext(nc) as tc, ExitStack() as top:
        const = top.enter_context(tc.tile_pool(name="const", bufs=1))
        rt_t = const.tile([128, 128], B16, tag="rt", name="rt")
        nc.sync.dma_start(rt_t[:], rT[:])
        id_t = const.tile([128, 128], B16, tag="id", name="id")
        nc.sync.dma_start(id_t[:], idT[:])
        bp_t = const.tile([128, 8], F32, tag="bp", name="bp")
        nc.sync.dma_start(bp_t[:], bpT[:])

        for _ in range(iters):
            with ExitStack() as it_stack:
                qkr = it_stack.enter_context(tc.tile_pool(name="qkr", bufs=1))
                qr_t = [qkr.tile([128, Nq], B16, tag=f"qr{m}", name=f"qr{m}")
                        for m in range(NPAIR)]
                kr_t = [qkr.tile([128, Nk], B16, tag=f"kr{m}", name=f"kr{m}")
                        for m in range(NPAIR)]
                vpool = it_stack.enter_context(tc.tile_pool(name="vpool",
                                                            bufs=1))
                v65 = vpool.tile([128, SC * 520], B16, tag="v65", name="v65")
                ptE = [vpool.tile([128, SC * 512], B16, tag=f"ptE{h}",
                                  name=f"ptE{h}") for h in range(NB_EARLY)]

                # ========== Phases A/B: Q/K projections + RoPE ==========
                with ExitStack() as ab:
                    stream = ab.enter_context(
                        tc.tile_pool(name="stream", bufs=2))
                    wstage = ab.enter_context(
                        tc.tile_pool(name="wstage", bufs=2))
                    qtbl = ab.enter_context(tc.tile_pool(name="qtbl",
                                                         bufs=1))
                    ktbl = ab.enter_context(tc.tile_pool(name="ktbl",
                                                         bufs=2))

                    with ExitStack() as abx:
                        rpsum = abx.enter_context(
                            tc.tile_pool(name="rpsum", bufs=2, space="PSUM"))
                        rope = abx.enter_context(
                            tc.tile_pool(name="rope", bufs=2))

                        def rope_block(x_psum, cos_ap, sin_ap, out_ap, width,
                                       evac):
                            """out = x*cos + rT.T @ (x*sin).  `evac` engine
                            evacuates the projection PSUM; DVE does muls and
                            the final add; PE applies the rotate matrix."""
                            xs = rope.tile([128, width], B16, tag="xs",
                                           name="xs")
                            if evac == "act":
                                nc.scalar.copy(xs[:], x_psum[:])
                            else:
                                nc.vector.tensor_copy(xs[:], x_psum[:])
                            tsin = rope.tile([128, width], B16, tag="tsin",
                                             name="tsin")
                            nc.vector.tensor_mul(tsin[:], xs[:], sin_ap)
                            prot = rpsum.tile([128, width], F32, tag="prot",
                                              name="prot")
                            nc.tensor.matmul(prot[:], rt_t[:], tsin[:],
                                             start=True, stop=True)
                            tcos = rope.tile([128, width], B16, tag="tcos",
                                             name="tcos")
                            nc.vector.tensor_mul(tcos[:], xs[:], cos_ap)
                            nc.vector.tensor_add(out_ap, tcos[:], prot[:])

                        def scores_exp(h, sc0, nsc, psc_pool, pt_ap):
                            """Scores for nsc key-chunks starting at sc0,
                            then one exp into pt_ap (cols 0..nsc*512)."""
                            p, r0 = head_slices(h)
                            psc = psc_pool.tile([128, nsc * 512], F32,
                                                tag="psc", name="psc")
                            for j in range(nsc):
                                sc = sc0 + j
                                nc.tensor.matmul(
                                    psc[:, j * 512:(j + 1) * 512],
                                    kr_t[p][r0:r0 + 64,
                                            sc * 128:(sc + 1) * 128],
                                    qr_t[p][r0:r0 + 64, :],
                                    start=True, stop=True)
                            nc.scalar.activation(pt_ap, psc[:], EXP,
                                                 scale=SCALE)

                        # ---- A: Q projection + RoPE ----
                        # per-ci DMA pairs so the first matmul starts after
                        # one 128-row slab instead of the full 8-slab load
                        with ExitStack() as ax:
                            apsum = ax.enter_context(
                                tc.tile_pool(name="apsum", bufs=4,
                                             space="PSUM"))
                            pq_t = [apsum.tile([128, Nq], F32, tag=f"pq{m}",
                                               name=f"pq{m}")
                                    for m in range(NPAIR)]
                            for ci in range(8):
                                wq_c = wstage.tile([128, CH], B16, tag="wqc",
                                                   name="wqc")
                                nc.sync.dma_start(
                                    wq_c[:], wqT[ci * 128:(ci + 1) * 128, :])
                                qt_c = stream.tile([128, Nq], B16, tag="qtc",
                                                   name="qtc")
                                nc.sync.dma_start(
                                    qt_c[:], qT[ci * 128:(ci + 1) * 128, :])
                                for m in range(NPAIR):
                                    nc.tensor.matmul(
                                        pq_t[m][:],
                                        wq_c[:, m * 128:(m + 1) * 128],
                                        qt_c[:],
                                        start=(ci == 0), stop=(ci == 7))
                            qc_t = qtbl.tile([128, Nq], B16, tag="qcos",
                                             name="qcos")
                            nc.sync.dma_start(qc_t[:], qcos[:])
                            qs_t = qtbl.tile([128, Nq], B16, tag="qsin",
                                             name="qsin")
                            nc.sync.dma_start(qs_t[:], qsin[:])
                            for m in range(NPAIR):
                                rope_block(pq_t[m], qc_t[:], qs_t[:],
                                           qr_t[m][:], Nq, "act")

                        # ---- B: K projection + RoPE + early scores/exp ----
                        wk_t = wstage.tile([128, 8 * CH], B16, tag="w",
                                           name="wk")
                        _ld3(nc, wk_t, wkT[:], CH)

                        with ExitStack() as bx:
                            pscE_pool = bx.enter_context(
                                tc.tile_pool(name="pscE", bufs=2,
                                             space="PSUM"))
                            for sbi in range(SB):
                                sl = slice(sbi * 512, (sbi + 1) * 512)
                                kc_t = ktbl.tile([128, 512], B16, tag="kcos",
                                                 name="kcos")
                                nc.sync.dma_start(kc_t[:], kcos[:, sl])
                                ks_t = ktbl.tile([128, 512], B16, tag="ksin",
                                                 name="ksin")
                                nc.sync.dma_start(ks_t[:], ksin[:, sl])
                                kt_t = stream.tile([128, 4096], B16, tag="s",
                                                   name="kt")
                                _ld3(nc, kt_t, kT[:, sl])
                                for m in range(NPAIR):
                                    pk = ppsum.tile([128, 512], F32,
                                                    tag="pq", name="pk")
                                    for ci in range(8):
                                        nc.tensor.matmul(
                                            pk[:],
                                            wk_t[:, ci * CH + m * 128:
                                                 ci * CH + (m + 1) * 128],
                                            kt_t[:, ci * 512:(ci + 1) * 512],
                                            start=(ci == 0), stop=(ci == 7))
                                    rope_block(pk, kc_t[:], ks_t[:],
                                               kr_t[m][:, sl], 512)
                                for h in range(NB_EARLY):
                                    for half in range(2):
                                        sc0 = sbi * 4 + half * 2
                                        scores_exp(
                                            h, sc0, 2, pscE_pool,
                                            ptE[h][:, sc0 * 512:
                                                   (sc0 + 2) * 512])

                # ===== pass0 + tail =====
                with ExitStack() as p0t:
                    ptL_pool = p0t.enter_context(tc.tile_pool(name="ptL",
                                                              bufs=3))
                    xpool = p0t.enter_context(tc.tile_pool(name="xpool",
                                                           bufs=1))
                    xq_t = [xpool.tile([128, CH], B16, tag=f"xq{qc}",
                                       name=f"xq{qc}") for qc in range(4)]
                    xt_t = [xpool.tile([128, Nq], B16, tag=f"xt{m}",
                                       name=f"xt{m}") for m in range(NPAIR)]
                    wp_t = xpool.tile([128, NPAIR * C], B16, tag="wp",
                                      name="wp")
                    nc.sync.dma_start(wp_t[:], wpT[:])
                    inv_pool = p0t.enter_context(tc.tile_pool(name="invp",
                                                              bufs=4))

                    # ones columns of v65 (col 64 of each 65-block)
                    nc.vector.memset(
                        v65[:].rearrange("p (n w) -> p n w", w=65)[:, :,
                                                                   64:65],
                        1.0)

                    ptL_t = {}

                    def scores_exp_mid(h, psc_pool):
                        ptl = ptL_pool.tile([128, SC * 512], B16, tag="pt",
                                            name="pt")
                        ptL_t[h] = ptl
                        p, r0 = head_slices(h)
                        for gi in range(8):
                            sc0 = gi * 2
                            psc = psc_pool.tile([128, 1024], F32, tag="psc",
                                                name="psc")
                            for j in range(2):
                                sc = sc0 + j
                                nc.tensor.matmul(
                                    psc[:, j * 512:(j + 1) * 512],
                                    kr_t[p][r0:r0 + 64,
                                            sc * 128:(sc + 1) * 128],
                                    qr_t[p][r0:r0 + 64, :],
                                    start=True, stop=True)
                            nc.scalar.activation(
                                ptl[:, sc0 * 512:(sc0 + 2) * 512], psc[:],
                                EXP, scale=SCALE)

                    # ---- pass0a: V projection + mid-head scores/exp ----
                    with ExitStack() as ph:
                        stream0 = ph.enter_context(
                            tc.tile_pool(name="stream0", bufs=2))
                        wstage0 = ph.enter_context(
                            tc.tile_pool(name="wstage0", bufs=1))
                        pv_pool = ph.enter_context(
                            tc.tile_pool(name="pv0", bufs=2, space="PSUM"))
                        pscM_pool = ph.enter_context(
                            tc.tile_pool(name="pscM", bufs=2, space="PSUM"))

                        wv_t = wstage0.tile([128, 8 * CH], B16, tag="w",
                                            name="wv")
                        _ld3(nc, wv_t, wvT[:], CH)
                        for sbi in range(SB):
                            sl = slice(sbi * 512, (sbi + 1) * 512)
                            vt_t = stream0.tile([128, 4096], B16, tag="s",
                                                name="vt")
                            _ld3(nc, vt_t, vT[:, sl])
                            for scj in range(4):
                                sc = sbi * 4 + scj
                                pv = pv_pool.tile([128, CH], F32, tag="pv",
                                                  name="pv")
                                for ci in range(8):
                                    nc.tensor.matmul(
                                        pv[:],
                                        vt_t[:, ci * 512 + scj * 128:
                                             ci * 512 + (scj + 1) * 128],
                                        wv_t[:, ci * CH:(ci + 1) * CH],
                                        start=(ci == 0), stop=(ci == 7))
                                dst = v65[:, sc * 520:(sc + 1) * 520
                                          ].rearrange(
                                              "p (n w) -> p n w",
                                              w=65)[:, :, 0:64]
                                nc.vector.tensor_copy(
                                    dst,
                                    pv[:].rearrange("p (n w) -> p n w",
                                                    w=64))
                            if sbi < NB_MID:
                                scores_exp_mid(NB_EARLY + sbi, pscM_pool)

                    # ---- pass0b: transposed PV, one bank per group ----
                    with ExitStack() as ph:
                        pvt_pool = ph.enter_context(
                            tc.tile_pool(name="pvt", bufs=4, space="PSUM"))
                        pscL_pool = ph.enter_context(
                            tc.tile_pool(name="pscL", bufs=2, space="PSUM"))

                        def pvt_head(h, pt_tile):
                            for qc in range(4):
                                acc = pvt_pool.tile(
                                    [128, 65], F32, tag="acc", name="acc",
                                    padded_shape=[128, 512])
                                for sc in range(SC):
                                    nc.tensor.matmul(
                                        acc[:],
                                        pt_tile[:, sc * 512 + qc * 128:
                                                sc * 512 + (qc + 1) * 128],
                                        v65[:, sc * 520 + h * 65:
                                            sc * 520 + (h + 1) * 65],
                                        start=(sc == 0), stop=(sc == SC - 1))
                                inv = inv_pool.tile([128, 1], F32, tag="inv",
                                                    name="inv")
                                nc.vector.reciprocal(inv[:], acc[:, 64:65])
                                nc.vector.tensor_scalar_mul(
                                    xq_t[qc][:, h * 64:(h + 1) * 64],
                                    acc[:, 0:64], inv[:])

                        for h in range(NB_EARLY):
                            pvt_head(h, ptE[h])
                        pvt_head(NB_EARLY, ptL_t[NB_EARLY])
                        # head 7's scores+exp reuse head 4's freed buffer
                        scores_exp_mid(7, pscL_pool)
                        for h in range(NB_EARLY + 1, NB_EARLY + NB_MID):
                            pvt_head(h, ptL_t[h])
                        pvt_head(7, ptL_t[7])

                    # ---- tail: transpose -> out proj ----
                    with ExitStack() as tl:
                        tp_pool = tl.enter_context(
                            tc.tile_pool(name="tp", bufs=4, space="PSUM"))
                        po_pool = tl.enter_context(
                            tc.tile_pool(name="po", bufs=2, space="PSUM"))
                        osb_pool = tl.enter_context(
                            tc.tile_pool(name="osb", bufs=2))
                        for qc in range(4):
                            for m in range(NPAIR):
                                tp = tp_pool.tile([128, 128], F32, tag="tp",
                                                  name="tp",
                                                  padded_shape=[128, 512])
                                nc.tensor.matmul(
                                    tp[:],
                                    xq_t[qc][:, m * 128:(m + 1) * 128],
                                    id_t[:], start=True, stop=True)
                                nc.scalar.copy(
                                    xt_t[m][:, qc * 128:(qc + 1) * 128],
                                    tp[:])
                        for j in range(8):
                            po = po_pool.tile([128, Nq], F32, tag="po",
                                              name="po")
                            for m in range(NPAIR):
                                nc.tensor.matmul(
                                    po[:],
                                    wp_t[:, m * C + j * 128:
                                         m * C + (j + 1) * 128],
                                    xt_t[m][:], start=(m == 0),
                                    stop=(m == NPAIR - 1))
                            osb = osb_pool.tile([128, Nq], F32, tag="osb",
                                                name="osb")
                            nc.vector.tensor_scalar_add(osb[:], po[:],
                                                        bp_t[:, j:j + 1])
                            nc.sync.dma_start(
                                outT[j * 128:(j + 1) * 128, :], osb[:])

    nc.compile()
    return nc


def prep_inputs(query, key, value, qpos, kpos, Wq, Wk, Wv, Wp, bp):
    """Build per-core input maps (8 cores: core = 2*b + g)."""
    bf16 = ml_dtypes.bfloat16
    invf = (1.0 / ROPE_BASE ** (np.arange(0, D, 2, dtype=np.float32) / D)
            ).astype(np.float32)
    rows64 = invf[np.arange(64) % 32]          # [64]

    R64 = np.zeros((64, 64), dtype=np.float32)
    for r in range(32):
        R64[r, r + 32] = -1.0
        R64[r + 32, r] = 1.0
    rT128 = np.zeros((128, 128), dtype=np.float32)
    rT128[0:64, 0:64] = R64.T
    rT128[64:128, 64:128] = R64.T

    def b(x):
        return np.ascontiguousarray(np.asarray(x, np.float32)).astype(bf16)

    in_maps = []
    for core in range(8):
        bi, g = core // 2, core % 2
        cols = slice(g * CH, (g + 1) * CH)
        qang = rows64[:, None] * np.asarray(qpos[bi], np.float32)[None, :]
        kang = rows64[:, None] * np.asarray(kpos[bi], np.float32)[None, :]
        Wg = np.asarray(Wp, np.float32)[:, cols]        # [C, CH]
        wp_pair = Wg.reshape(C, NPAIR, 128).transpose(2, 1, 0).reshape(
            128, NPAIR * C)
        m = {
            "qT": b(np.asarray(query[bi], np.float32).T),
            "kT": b(np.asarray(key[bi], np.float32).T),
            "vT": b(np.asarray(value[bi], np.float32).T),
            "wqT": b(np.asarray(Wq, np.float32)[cols, :].T),
            "wkT": b(np.asarray(Wk, np.float32)[cols, :].T),
            "wvT": b(np.asarray(Wv, np.float32)[cols, :].T),
            "wpT": b(wp_pair),
            "bpT": (np.ascontiguousarray(
                        np.asarray(bp, np.float32).reshape(8, 128).T)
                    if g == 0 else np.zeros((128, 8), np.float32)),
            "qcos": b(np.tile(np.cos(qang), (2, 1))),
            "qsin": b(np.tile(np.sin(qang), (2, 1))),
            "kcos": b(np.tile(np.cos(kang), (2, 1))),
            "ksin": b(np.tile(np.sin(kang), (2, 1))),
            "rT": b(rT128),
            "idT": b(np.eye(128, dtype=np.float32)),
        }
        in_maps.append(m)
    return in_maps


_NC_CACHE = {}


def _get_nc(iters=1):
    if iters not in _NC_CACHE:
        _NC_CACHE[iters] = build_nc(iters)
    return _NC_CACHE[iters]


def kernel(query, key, value, qpos, kpos, Wq, Wk, Wv, Wp, bp):
    from concourse.bass_utils import run_bass_kernel_spmd

    nc = _get_nc()
    in_maps = prep_inputs(query, key, value, qpos, kpos, Wq, Wk, Wv, Wp, bp)
    res = run_bass_kernel_spmd(nc, in_maps, list(range(8)))
    out = np.zeros((B, Nq, C), dtype=np.float32)
    for core in range(8):
        out[core // 2] += res.results[core]["outT"].T
    return out


# revision 25
# speedup vs baseline: 1.3779x; 1.1016x over previous
"""CrossAttention (RoPE, 16 heads, C=1024) Trainium2 Bass kernel.

Sharding: DP over batch (4) x TP over heads (2 groups of 8) = 8 cores.
Each core computes, for its (batch b, head-group g):
  Q/K/V projections (column-parallel), RoPE, scores, exp (softmax without
  max-subtraction; logits are bounded), transposed PV accumulation with an
  appended ones-column for the row-sums, late normalization, and the
  row-parallel output projection producing a partial out^T.  The host sums
  the two head-group partials.

All data is bf16 in SBUF (f32 PSUM accumulation), halving DMA traffic and
enabling small-moving-dim matmuls at full rate.

Pipeline (engine balance):
  A: Q proj + RoPE.
  B: K proj + RoPE, with scores+exp for heads 0..3 interleaved per key
     block so the ACT engine starts the softmax early (probs are saved
     in SBUF until pass0b).
  pass0a: V projection streamed per 128-key chunk into v65; heads 4..6
     run scores+exp here into rotating prob buffers.
  pass0b: transposed PV (out [q, 64ch+1ones], moving dim 65) — one PSUM
     accumulation group per (head, q-chunk), one bank per group (PSUM
     start_tensor_calc zeroes a whole 2KB bank, so concurrent groups must
     not share banks).  Head 7's scores+exp overlap this phase.
     Normalization happens per group: DVE reciprocal of the ones column,
     Pool tensor_scalar multiply into xq.
  tail: PE transposes xq back to [ch, q], paired output projection
     (contraction 128 = head pair), bias add, DMA out.

Layout notes (per core):
  qT  [C, Nq]   kT [C, Nk]   vT [C, Nk]      (activations, transposed, bf16)
  wqT/wkT/wvT [C, 512]   wpT [128, 4*C]      (weight slices, host-prepped)
  v65 [128, SC*520]: per key-chunk sc, per head h a [128, 65] block of
  64 V-channels plus a ones column.
  rope tables [128, N] bf16 (64-row table duplicated; rows use
  inv_freq[r%32]);  rT [128, 128] block-diag rotate-half matrix.
  RoPE identity:  rope(x) = x*cos + rT.T @ (x*sin).
"""

import sys

if "/opt/trn_rl_repo" not in sys.path:
    sys.path.insert(0, "/opt/trn_rl_repo")

import numpy as np
import ml_dtypes
from contextlib import ExitStack

import concourse.bass as bass
import concourse.tile as tile
from concourse import bacc, mybir

F32 = mybir.dt.float32
B16 = mybir.dt.bfloat16
EXP = mybir.ActivationFunctionType.Exp

# problem constants
B, Nq, Nk, C = 4, 512, 2048, 1024
H, D = 16, 64
HL = 8            # heads per core
CH = HL * D       # 512 local channels
NPAIR = HL // 2   # 4 pair-chunks of 128 channels
SC = Nk // 128    # 16 key chunks of 128
SB = Nk // 512    # 4 key blocks of 512
ROPE_BASE = 10000.0
SCALE = float(D) ** -0.5

NB_EARLY = 3      # heads whose scores+exp run during phase B
NB_MID = 3        # heads whose scores+exp run during pass0a (rotating bufs)


def _ld3(nc, dst, src_2d, width=512):
    """One DMA loading a [N*128, width] DRAM region into a [128, N*width]
    tile (row-chunk ci lands at columns [ci*width, (ci+1)*width))."""
    nc.sync.dma_start(
        dst[:].rearrange("p (a s) -> p a s", s=width),
        src_2d.rearrange("(a p) s -> p a s", p=128))


def build_nc(iters: int = 1):
    nc = bacc.Bacc("TRN2", target_bir_lowering=False, debug=False)

    qT = nc.dram_tensor("qT", [C, Nq], B16, kind="ExternalInput")
    kT = nc.dram_tensor("kT", [C, Nk], B16, kind="ExternalInput")
    vT = nc.dram_tensor("vT", [C, Nk], B16, kind="ExternalInput")
    wqT = nc.dram_tensor("wqT", [C, CH], B16, kind="ExternalInput")
    wkT = nc.dram_tensor("wkT", [C, CH], B16, kind="ExternalInput")
    wvT = nc.dram_tensor("wvT", [C, CH], B16, kind="ExternalInput")
    wpT = nc.dram_tensor("wpT", [128, NPAIR * C], B16, kind="ExternalInput")
    bpT = nc.dram_tensor("bpT", [128, 8], F32, kind="ExternalInput")
    qcos = nc.dram_tensor("qcos", [128, Nq], B16, kind="ExternalInput")
    qsin = nc.dram_tensor("qsin", [128, Nq], B16, kind="ExternalInput")
    kcos = nc.dram_tensor("kcos", [128, Nk], B16, kind="ExternalInput")
    ksin = nc.dram_tensor("ksin", [128, Nk], B16, kind="ExternalInput")
    rT = nc.dram_tensor("rT", [128, 128], B16, kind="ExternalInput")
    idT = nc.dram_tensor("idT", [128, 128], B16, kind="ExternalInput")
    outT = nc.dram_tensor("outT", [C, Nq], B16, kind="ExternalOutput")

    def head_slices(h):
        """kr/qr pair index and row offset for local head h."""
        return h // 2, 64 * (h % 2)

    with tile.TileContext(nc) as tc, ExitStack() as top:
        const = top.enter_context(tc.tile_pool(name="const", bufs=1))
        rt_t = const.tile([128, 128], B16, tag="rt", name="rt")
        id_t = const.tile([128, 128], B16, tag="id", name="id")
        bp_t = const.tile([128, 8], F32, tag="bp", name="bp")
        const_loaded = False

        for _ in range(iters):
            with ExitStack() as it_stack:
                qkr = it_stack.enter_context(tc.tile_pool(name="qkr", bufs=1))
                qr_t = [qkr.tile([128, Nq], B16, tag=f"qr{m}", name=f"qr{m}")
                        for m in range(NPAIR)]
                kr_t = [qkr.tile([128, Nk], B16, tag=f"kr{m}", name=f"kr{m}")
                        for m in range(NPAIR)]
                vpool = it_stack.enter_context(tc.tile_pool(name="vpool",
                                                            bufs=1))
                v65 = vpool.tile([128, SC * 520], B16, tag="v65", name="v65")
                ptE = [vpool.tile([128, SC * 512], B16, tag=f"ptE{h}",
                                  name=f"ptE{h}") for h in range(NB_EARLY)]

                # ========== Phases A/B: Q/K projections + RoPE ==========
                # Deep pipeline: the projection PSUM tiles are shared
                # between A and B (4 banks); RoPE runs in two stages so
                # the PSUM is evacuated early (stage1) and the rotate/add
                # (stage2) is emitted inside the NEXT block, interleaved
                # with its projection ci-groups.  Early-head scores for
                # block i-1 are also embedded between block i's ci-groups
                # so exp latency on ACT never throttles the PE stream.
                trail_scores = []
                with ExitStack() as ab:
                    stream = ab.enter_context(
                        tc.tile_pool(name="stream", bufs=2))
                    wstage = ab.enter_context(
                        tc.tile_pool(name="wstage", bufs=2))
                    qtbl = ab.enter_context(tc.tile_pool(name="qtbl",
                                                         bufs=1))
                    ktbl = ab.enter_context(tc.tile_pool(name="ktbl",
                                                         bufs=2))

                    with ExitStack() as abx:
                        projpsum = abx.enter_context(
                            tc.tile_pool(name="projpsum", bufs=1,
                                         space="PSUM"))
                        rpsum = abx.enter_context(
                            tc.tile_pool(name="rpsum", bufs=2, space="PSUM"))
                        pscE_pool = abx.enter_context(
                            tc.tile_pool(name="pscE", bufs=2, space="PSUM"))
                        rope = abx.enter_context(
                            tc.tile_pool(name="rope", bufs=2))

                        def proj_tiles(width):
                            return [projpsum.tile([128, width], F32,
                                                  tag=f"pj{m}",
                                                  name=f"pj{m}",
                                                  padded_shape=[128, 512])
                                    for m in range(NPAIR)]

                        def rope_stage1(pj_t, width, n_act):
                            """Evacuate projection PSUM to bf16 (frees the
                            banks for the next block).  First n_act pairs
                            go through ACT, the rest through DVE."""
                            xs_l = []
                            for m in range(NPAIR):
                                xs = rope.tile([128, width], B16,
                                               tag=f"xs{m}", name=f"xs{m}")
                                if m < n_act:
                                    nc.scalar.copy(xs[:], pj_t[m][:])
                                else:
                                    nc.vector.tensor_copy(xs[:], pj_t[m][:])
                                xs_l.append(xs)
                            return xs_l

                        def rope_tsins(pend):
                            xs_l, cos_ap, sin_ap, outs, width = pend
                            ts_l = []
                            for m in range(NPAIR):
                                tsin = rope.tile([128, width], B16,
                                                 tag="tsin", name="tsin",
                                                 bufs=4)
                                nc.vector.tensor_mul(tsin[:], xs_l[m][:],
                                                     sin_ap)
                                ts_l.append(tsin)
                            return ts_l

                        def rope_rot(pend, ts_l, m):
                            """Stage 2 for pair m: rotate matmul + cos mul
                            + add into qr/kr."""
                            xs_l, cos_ap, sin_ap, outs, width = pend
                            prot = rpsum.tile([128, width], F32, tag="prot",
                                              name="prot",
                                              padded_shape=[128, 512])
                            nc.tensor.matmul(prot[:], rt_t[:], ts_l[m][:],
                                             start=True, stop=True)
                            tcos = rope.tile([128, width], B16, tag="tcos",
                                             name="tcos")
                            nc.vector.tensor_mul(tcos[:], xs_l[m][:], cos_ap)
                            nc.vector.tensor_add(outs[m], tcos[:], prot[:])

                        def scores_exp(h, sc, psc_pool, tag="psc"):
                            """Scores for one key-chunk, exp into ptE."""
                            p, r0 = head_slices(h)
                            psc = psc_pool.tile([128, 512], F32,
                                                tag=tag, name="psc")
                            nc.tensor.matmul(
                                psc[:],
                                kr_t[p][r0:r0 + 64,
                                        sc * 128:(sc + 1) * 128],
                                qr_t[p][r0:r0 + 64, :],
                                start=True, stop=True)
                            nc.scalar.activation(
                                ptE[h][:, sc * 512:(sc + 1) * 512], psc[:],
                                EXP, scale=SCALE)

                        # ---- A: Q projection, per-slab streamed ----
                        with ExitStack() as ax:
                            apool = ax.enter_context(
                                tc.tile_pool(name="apool", bufs=4))
                            pq_t = proj_tiles(Nq)
                            for ci in range(8):
                                wq_c = apool.tile([128, CH], B16, tag="wqc",
                                                  name="wqc")
                                nc.sync.dma_start(
                                    wq_c[:], wqT[ci * 128:(ci + 1) * 128, :])
                                qt_c = apool.tile([128, Nq], B16, tag="qtc",
                                                  name="qtc")
                                nc.sync.dma_start(
                                    qt_c[:], qT[ci * 128:(ci + 1) * 128, :])
                                for m in range(NPAIR):
                                    nc.tensor.matmul(
                                        pq_t[m][:],
                                        wq_c[:, m * 128:(m + 1) * 128],
                                        qt_c[:],
                                        start=(ci == 0), stop=(ci == 7))
                            qc_t = qtbl.tile([128, Nq], B16, tag="qcos",
                                             name="qcos")
                            nc.sync.dma_start(qc_t[:], qcos[:])
                            qs_t = qtbl.tile([128, Nq], B16, tag="qsin",
                                             name="qsin")
                            nc.sync.dma_start(qs_t[:], qsin[:])
                            if not const_loaded:
                                const_loaded = True
                                nc.sync.dma_start(rt_t[:], rT[:])
                                nc.sync.dma_start(id_t[:], idT[:])
                                nc.sync.dma_start(bp_t[:], bpT[:])
                            xs_a = rope_stage1(pq_t, Nq, 2)
                            pending = (xs_a, qc_t[:], qs_t[:],
                                       [qr_t[m][:] for m in range(NPAIR)],
                                       Nq)

                        # ---- B: K projection blocks, deep-pipelined ----
                        wk_t = wstage.tile([128, 8 * CH], B16, tag="w",
                                           name="wk")
                        kt0_t = stream.tile([128, 4096], B16, tag="s",
                                            name="kt0")
                        for ci in range(8):
                            nc.sync.dma_start(
                                wk_t[:, ci * CH:(ci + 1) * CH],
                                wkT[ci * 128:(ci + 1) * 128, :])
                            nc.sync.dma_start(
                                kt0_t[:, ci * 512:(ci + 1) * 512],
                                kT[ci * 128:(ci + 1) * 128, 0:512])

                        score_q = []      # deferred (h, sc) score/exp work
                        for sbi in range(SB):
                            sl = slice(sbi * 512, (sbi + 1) * 512)
                            kc_t = ktbl.tile([128, 512], B16, tag="kcos",
                                             name="kcos")
                            nc.sync.dma_start(kc_t[:], kcos[:, sl])
                            ks_t = ktbl.tile([128, 512], B16, tag="ksin",
                                             name="ksin")
                            nc.sync.dma_start(ks_t[:], ksin[:, sl])
                            if sbi == 0:
                                kt_t = kt0_t
                            else:
                                kt_t = stream.tile([128, 4096], B16,
                                                   tag="s", name="kt")
                                _ld3(nc, kt_t, kT[:, sl])
                            pk_t = proj_tiles(512)
                            ts_l = rope_tsins(pending)
                            for ci in range(8):
                                for m in range(NPAIR):
                                    nc.tensor.matmul(
                                        pk_t[m][:],
                                        wk_t[:, ci * CH + m * 128:
                                             ci * CH + (m + 1) * 128],
                                        kt_t[:, ci * 512:(ci + 1) * 512],
                                        start=(ci == 0), stop=(ci == 7))
                                if 1 <= ci <= 4:
                                    rope_rot(pending, ts_l, ci - 1)
                                if ci >= 2:
                                    for _ in range(2):
                                        if score_q:
                                            h, sc = score_q.pop(0)
                                            scores_exp(h, sc, pscE_pool)
                            # this block's rope stage 1 + bookkeeping
                            xs_b = rope_stage1(pk_t, 512, 0)
                            pending = (xs_b, kc_t[:], ks_t[:],
                                       [kr_t[m][:, sl]
                                        for m in range(NPAIR)], 512)
                            for h in range(NB_EARLY):
                                for scj in range(4):
                                    score_q.append((h, sbi * 4 + scj))
                        # final block's rotate/add + leftover scores
                        ts_l = rope_tsins(pending)
                        for m in range(NPAIR):
                            rope_rot(pending, ts_l, m)
                        while len(score_q) > NB_EARLY * 4:
                            h, sc = score_q.pop(0)
                            scores_exp(h, sc, pscE_pool)
                        trail_scores = score_q   # last block: into pass0a

                # ===== pass0 + tail =====
                with ExitStack() as p0t:
                    ptL_pool = p0t.enter_context(tc.tile_pool(name="ptL",
                                                              bufs=3))
                    xpool = p0t.enter_context(tc.tile_pool(name="xpool",
                                                           bufs=1))
                    xq_t = [xpool.tile([128, CH], B16, tag=f"xq{qc}",
                                       name=f"xq{qc}") for qc in range(4)]
                    xt_t = [xpool.tile([128, Nq], B16, tag=f"xt{m}",
                                       name=f"xt{m}") for m in range(NPAIR)]
                    wp_t = xpool.tile([128, NPAIR * C], B16, tag="wp",
                                      name="wp")
                    nc.sync.dma_start(wp_t[:], wpT[:])
                    inv_pool = p0t.enter_context(tc.tile_pool(name="invp",
                                                              bufs=4))

                    # ones columns of v65 (col 64 of each 65-block)
                    nc.vector.memset(
                        v65[:].rearrange("p (n w) -> p n w", w=65)[:, :,
                                                                   64:65],
                        1.0)

                    ptL_t = {}

                    def late_group(h, gi, psc_pool):
                        """One 2-chunk scores+exp group for a mid/late
                        head, saved into its rotating prob buffer."""
                        if h not in ptL_t:
                            ptL_t[h] = ptL_pool.tile([128, SC * 512], B16,
                                                     tag="pt", name="pt")
                        ptl = ptL_t[h]
                        p, r0 = head_slices(h)
                        sc0 = gi * 2
                        psc = psc_pool.tile([128, 1024], F32, tag="psc",
                                            name="psc")
                        for j in range(2):
                            sc = sc0 + j
                            nc.tensor.matmul(
                                psc[:, j * 512:(j + 1) * 512],
                                kr_t[p][r0:r0 + 64,
                                        sc * 128:(sc + 1) * 128],
                                qr_t[p][r0:r0 + 64, :],
                                start=True, stop=True)
                        nc.scalar.activation(
                            ptl[:, sc0 * 512:(sc0 + 2) * 512], psc[:],
                            EXP, scale=SCALE)

                    # ---- pass0a: V projection + mid-head scores/exp ----
                    with ExitStack() as ph:
                        stream0 = ph.enter_context(
                            tc.tile_pool(name="stream0", bufs=2))
                        wstage0 = ph.enter_context(
                            tc.tile_pool(name="wstage0", bufs=1))
                        pv_pool = ph.enter_context(
                            tc.tile_pool(name="pv0", bufs=2, space="PSUM"))
                        pscM_pool = ph.enter_context(
                            tc.tile_pool(name="pscM", bufs=2, space="PSUM"))

                        def trail_score(h, sc):
                            """Leftover early-head score from phase B."""
                            p, r0 = head_slices(h)
                            psc = pscM_pool.tile([128, 512], F32,
                                                 tag="psc1", name="psc1")
                            nc.tensor.matmul(
                                psc[:],
                                kr_t[p][r0:r0 + 64,
                                        sc * 128:(sc + 1) * 128],
                                qr_t[p][r0:r0 + 64, :],
                                start=True, stop=True)
                            nc.scalar.activation(
                                ptE[h][:, sc * 512:(sc + 1) * 512], psc[:],
                                EXP, scale=SCALE)

                        mid_q = [(h, gi)
                                 for h in range(NB_EARLY, NB_EARLY + NB_MID)
                                 for gi in range(8)]
                        wv_t = wstage0.tile([128, 8 * CH], B16, tag="w",
                                            name="wv")
                        _ld3(nc, wv_t, wvT[:], CH)
                        for sbi in range(SB):
                            sl = slice(sbi * 512, (sbi + 1) * 512)
                            vt_t = stream0.tile([128, 4096], B16, tag="s",
                                                name="vt")
                            _ld3(nc, vt_t, vT[:, sl])
                            for scj in range(4):
                                sc = sbi * 4 + scj
                                pv = pv_pool.tile([128, CH], F32, tag="pv",
                                                  name="pv")
                                for ci in range(8):
                                    nc.tensor.matmul(
                                        pv[:],
                                        vt_t[:, ci * 512 + scj * 128:
                                             ci * 512 + (scj + 1) * 128],
                                        wv_t[:, ci * CH:(ci + 1) * CH],
                                        start=(ci == 0), stop=(ci == 7))
                                dst = v65[:, sc * 520:(sc + 1) * 520
                                          ].rearrange(
                                              "p (n w) -> p n w",
                                              w=65)[:, :, 0:64]
                                nc.vector.tensor_copy(
                                    dst,
                                    pv[:].rearrange("p (n w) -> p n w",
                                                    w=64))
                                if trail_scores:
                                    trail_score(*trail_scores.pop(0))
                                for _ in range(2):
                                    if mid_q:
                                        late_group(*mid_q.pop(0),
                                                   pscM_pool)

                    # ---- pass0b: transposed PV, one bank per group;
                    # heads 6/7 run scores+exp between PV groups ----
                    with ExitStack() as ph:
                        pvt_pool = ph.enter_context(
                            tc.tile_pool(name="pvt", bufs=4, space="PSUM"))
                        pscL_pool = ph.enter_context(
                            tc.tile_pool(name="pscL", bufs=2, space="PSUM"))

                        def pvt_head(h, pt_tile):
                            for qc in range(4):
                                acc = pvt_pool.tile(
                                    [128, 65], F32, tag="acc", name="acc",
                                    padded_shape=[128, 512])
                                for sc in range(SC):
                                    nc.tensor.matmul(
                                        acc[:],
                                        pt_tile[:, sc * 512 + qc * 128:
                                                sc * 512 + (qc + 1) * 128],
                                        v65[:, sc * 520 + h * 65:
                                            sc * 520 + (h + 1) * 65],
                                        start=(sc == 0), stop=(sc == SC - 1))
                                inv = inv_pool.tile([128, 1], F32, tag="inv",
                                                    name="inv")
                                nc.vector.reciprocal(inv[:], acc[:, 64:65])
                                nc.vector.tensor_scalar_mul(
                                    xq_t[qc][:, h * 64:(h + 1) * 64],
                                    acc[:, 0:64], inv[:])

                        late_q = [(6, gi) for gi in range(8)] + \
                                 [(7, gi) for gi in range(8)]
                        pvt_head(NB_EARLY, ptL_t[NB_EARLY])
                        plan = [(0, 3), (1, 3), (2, 2), (4, 3), (5, 3),
                                (6, 2), (7, 0)]
                        for h, ng in plan:
                            src_t = ptE[h] if h < NB_EARLY else ptL_t[h]
                            pvt_head(h, src_t)
                            for _ in range(ng):
                                if late_q:
                                    late_group(*late_q.pop(0), pscL_pool)

                    # ---- tail: transpose -> out proj ----
                    with ExitStack() as tl:
                        tp_pool = tl.enter_context(
                            tc.tile_pool(name="tp", bufs=4, space="PSUM"))
                        po_pool = tl.enter_context(
                            tc.tile_pool(name="po", bufs=4, space="PSUM"))
                        osb_pool = tl.enter_context(
                            tc.tile_pool(name="osb", bufs=3))
                        for qc in range(4):
                            for m in range(NPAIR):
                                tp = tp_pool.tile([128, 128], F32, tag="tp",
                                                  name="tp",
                                                  padded_shape=[128, 512])
                                nc.tensor.matmul(
                                    tp[:],
                                    xq_t[qc][:, m * 128:(m + 1) * 128],
                                    id_t[:], start=True, stop=True)
                                if m < 2:
                                    nc.scalar.copy(
                                        xt_t[m][:, qc * 128:(qc + 1) * 128],
                                        tp[:])
                                else:
                                    nc.vector.tensor_copy(
                                        xt_t[m][:, qc * 128:(qc + 1) * 128],
                                        tp[:])
                        for j in range(8):
                            po = po_pool.tile([128, Nq], F32, tag="po",
                                              name="po")
                            for m in range(NPAIR):
                                nc.tensor.matmul(
                                    po[:],
                                    wp_t[:, m * C + j * 128:
                                         m * C + (j + 1) * 128],
                                    xt_t[m][:], start=(m == 0),
                                    stop=(m == NPAIR - 1))
                            osb = osb_pool.tile([128, Nq], B16, tag="osb",
                                                name="osb")
                            nc.scalar.activation(
                                osb[:], po[:],
                                mybir.ActivationFunctionType.Identity,
                                bias=bp_t[:, j:j + 1])
                            nc.sync.dma_start(
                                outT[j * 128:(j + 1) * 128, :], osb[:])

    nc.compile()
    return nc


def prep_inputs(query, key, value, qpos, kpos, Wq, Wk, Wv, Wp, bp):
    """Build per-core input maps (8 cores: core = 2*b + g)."""
    bf16 = ml_dtypes.bfloat16
    invf = (1.0 / ROPE_BASE ** (np.arange(0, D, 2, dtype=np.float32) / D)
            ).astype(np.float32)
    rows64 = invf[np.arange(64) % 32]          # [64]

    R64 = np.zeros((64, 64), dtype=np.float32)
    for r in range(32):
        R64[r, r + 32] = -1.0
        R64[r + 32, r] = 1.0
    rT128 = np.zeros((128, 128), dtype=np.float32)
    rT128[0:64, 0:64] = R64.T
    rT128[64:128, 64:128] = R64.T

    def b(x):
        return np.ascontiguousarray(np.asarray(x, np.float32)).astype(bf16)

    in_maps = []
    for core in range(8):
        bi, g = core // 2, core % 2
        cols = slice(g * CH, (g + 1) * CH)
        qang = rows64[:, None] * np.asarray(qpos[bi], np.float32)[None, :]
        kang = rows64[:, None] * np.asarray(kpos[bi], np.float32)[None, :]
        Wg = np.asarray(Wp, np.float32)[:, cols]        # [C, CH]
        wp_pair = Wg.reshape(C, NPAIR, 128).transpose(2, 1, 0).reshape(
            128, NPAIR * C)
        m = {
            "qT": b(np.asarray(query[bi], np.float32).T),
            "kT": b(np.asarray(key[bi], np.float32).T),
            "vT": b(np.asarray(value[bi], np.float32).T),
            "wqT": b(np.asarray(Wq, np.float32)[cols, :].T),
            "wkT": b(np.asarray(Wk, np.float32)[cols, :].T),
            "wvT": b(np.asarray(Wv, np.float32)[cols, :].T),
            "wpT": b(wp_pair),
            "bpT": (np.ascontiguousarray(
                        np.asarray(bp, np.float32).reshape(8, 128).T)
                    if g == 0 else np.zeros((128, 8), np.float32)),
            "qcos": b(np.tile(np.cos(qang), (2, 1))),
            "qsin": b(np.tile(np.sin(qang), (2, 1))),
            "kcos": b(np.tile(np.cos(kang), (2, 1))),
            "ksin": b(np.tile(np.sin(kang), (2, 1))),
            "rT": b(rT128),
            "idT": b(np.eye(128, dtype=np.float32)),
        }
        in_maps.append(m)
    return in_maps


_NC_CACHE = {}


def _get_nc(iters=1):
    if iters not in _NC_CACHE:
        _NC_CACHE[iters] = build_nc(iters)
    return _NC_CACHE[iters]


def kernel(query, key, value, qpos, kpos, Wq, Wk, Wv, Wp, bp):
    from concourse.bass_utils import run_bass_kernel_spmd

    nc = _get_nc()
    in_maps = prep_inputs(query, key, value, qpos, kpos, Wq, Wk, Wv, Wp, bp)
    res = run_bass_kernel_spmd(nc, in_maps, list(range(8)))
    out = np.zeros((B, Nq, C), dtype=np.float32)
    for core in range(8):
        out[core // 2] += np.asarray(res.results[core]["outT"],
                             dtype=np.float32).T
    return out


# revision 31
# speedup vs baseline: 1.4274x; 1.0359x over previous
"""CrossAttention (RoPE, 16 heads, C=1024) Trainium2 Bass kernel.

Sharding: DP over batch (4) x TP over heads (2 groups of 8) = 8 cores.
Each core computes, for its (batch b, head-group g):
  Q/K/V projections (column-parallel), RoPE, scores, exp (softmax without
  max-subtraction; logits are bounded), transposed PV accumulation with an
  appended ones-column for the row-sums, late normalization, and the
  row-parallel output projection producing a partial out^T.  The host sums
  the two head-group partials.

All data is bf16 in SBUF (f32 PSUM accumulation), halving DMA traffic and
enabling small-moving-dim matmuls at full rate.

Pipeline (engine balance):
  A: Q proj + RoPE.
  B: K proj + RoPE, with scores+exp for heads 0..3 interleaved per key
     block so the ACT engine starts the softmax early (probs are saved
     in SBUF until pass0b).
  pass0a: V projection streamed per 128-key chunk into v65; heads 4..6
     run scores+exp here into rotating prob buffers.
  pass0b: transposed PV (out [q, 64ch+1ones], moving dim 65) — one PSUM
     accumulation group per (head, q-chunk), one bank per group (PSUM
     start_tensor_calc zeroes a whole 2KB bank, so concurrent groups must
     not share banks).  Head 7's scores+exp overlap this phase.
     Normalization happens per group: DVE reciprocal of the ones column,
     Pool tensor_scalar multiply into xq.
  tail: PE transposes xq back to [ch, q], paired output projection
     (contraction 128 = head pair), bias add, DMA out.

Layout notes (per core):
  qT  [C, Nq]   kT [C, Nk]   vT [C, Nk]      (activations, transposed, bf16)
  wqT/wkT/wvT [C, 512]   wpT [128, 4*C]      (weight slices, host-prepped)
  v65 [128, SC*520]: per key-chunk sc, per head h a [128, 65] block of
  64 V-channels plus a ones column.
  rope tables [128, N] bf16 (64-row table duplicated; rows use
  inv_freq[r%32]);  rT [128, 128] block-diag rotate-half matrix.
  RoPE identity:  rope(x) = x*cos + rT.T @ (x*sin).
"""

import sys

if "/opt/trn_rl_repo" not in sys.path:
    sys.path.insert(0, "/opt/trn_rl_repo")

import numpy as np
import ml_dtypes
from contextlib import ExitStack

import concourse.bass as bass
import concourse.tile as tile
from concourse import bacc, mybir

F32 = mybir.dt.float32
B16 = mybir.dt.bfloat16
EXP = mybir.ActivationFunctionType.Exp

# problem constants
B, Nq, Nk, C = 4, 512, 2048, 1024
H, D = 16, 64
HL = 8            # heads per core
CH = HL * D       # 512 local channels
NPAIR = HL // 2   # 4 pair-chunks of 128 channels
SC = Nk // 128    # 16 key chunks of 128
SB = Nk // 512    # 4 key blocks of 512
ROPE_BASE = 10000.0
SCALE = float(D) ** -0.5

NB_EARLY = 3      # heads whose scores+exp run during phase B
NB_MID = 3        # heads whose scores+exp run during pass0a (rotating bufs)


def _ld3(nc, dst, src_2d, width=512):
    """One DMA loading a [N*128, width] DRAM region into a [128, N*width]
    tile (row-chunk ci lands at columns [ci*width, (ci+1)*width))."""
    nc.sync.dma_start(
        dst[:].rearrange("p (a s) -> p a s", s=width),
        src_2d.rearrange("(a p) s -> p a s", p=128))


def build_nc(iters: int = 1):
    nc = bacc.Bacc("TRN2", target_bir_lowering=False, debug=False)

    qT = nc.dram_tensor("qT", [C, Nq], B16, kind="ExternalInput")
    kT = nc.dram_tensor("kT", [C, Nk], B16, kind="ExternalInput")
    vT = nc.dram_tensor("vT", [C, Nk], B16, kind="ExternalInput")
    wqT = nc.dram_tensor("wqT", [C, CH], B16, kind="ExternalInput")
    wkT = nc.dram_tensor("wkT", [C, CH], B16, kind="ExternalInput")
    wvT = nc.dram_tensor("wvT", [C, CH], B16, kind="ExternalInput")
    wpT = nc.dram_tensor("wpT", [128, NPAIR * C], B16, kind="ExternalInput")
    bpT = nc.dram_tensor("bpT", [128, 8], F32, kind="ExternalInput")
    qcos = nc.dram_tensor("qcos", [128, Nq], B16, kind="ExternalInput")
    qsin = nc.dram_tensor("qsin", [128, Nq], B16, kind="ExternalInput")
    kcos = nc.dram_tensor("kcos", [128, Nk], B16, kind="ExternalInput")
    ksin = nc.dram_tensor("ksin", [128, Nk], B16, kind="ExternalInput")
    rT = nc.dram_tensor("rT", [128, 128], B16, kind="ExternalInput")
    idT = nc.dram_tensor("idT", [128, 128], B16, kind="ExternalInput")
    outT = nc.dram_tensor("outT", [C, Nq], B16, kind="ExternalOutput")

    def head_slices(h):
        """kr/qr pair index and row offset for local head h."""
        return h // 2, 64 * (h % 2)

    with tile.TileContext(nc) as tc, ExitStack() as top:
        const = top.enter_context(tc.tile_pool(name="const", bufs=1))
        rt_t = const.tile([128, 128], B16, tag="rt", name="rt")
        id_t = const.tile([128, 128], B16, tag="id", name="id")
        bp_t = const.tile([128, 8], F32, tag="bp", name="bp")
        const_loaded = False

        for _ in range(iters):
            with ExitStack() as it_stack:
                qkr = it_stack.enter_context(tc.tile_pool(name="qkr", bufs=1))
                qr_t = [qkr.tile([128, Nq], B16, tag=f"qr{m}", name=f"qr{m}")
                        for m in range(NPAIR)]
                kr_t = [qkr.tile([128, Nk], B16, tag=f"kr{m}", name=f"kr{m}")
                        for m in range(NPAIR)]
                vpool = it_stack.enter_context(tc.tile_pool(name="vpool",
                                                            bufs=1))
                v65 = vpool.tile([128, SC * 520], B16, tag="v65", name="v65")
                ptE = [vpool.tile([128, SC * 512], B16, tag=f"ptE{h}",
                                  name=f"ptE{h}") for h in range(NB_EARLY)]

                # ========== Phases A/B: Q/K projections + RoPE ==========
                # Deep pipeline: the projection PSUM tiles are shared
                # between A and B (4 banks); RoPE runs in two stages so
                # the PSUM is evacuated early (stage1) and the rotate/add
                # (stage2) is emitted inside the NEXT block, interleaved
                # with its projection ci-groups.  Early-head scores for
                # block i-1 are also embedded between block i's ci-groups
                # so exp latency on ACT never throttles the PE stream.
                trail_scores = []
                with ExitStack() as ab:
                    stream = ab.enter_context(
                        tc.tile_pool(name="stream", bufs=2))
                    wstage = ab.enter_context(
                        tc.tile_pool(name="wstage", bufs=2))
                    qtbl = ab.enter_context(tc.tile_pool(name="qtbl",
                                                         bufs=1))
                    ktbl = ab.enter_context(tc.tile_pool(name="ktbl",
                                                         bufs=2))

                    with ExitStack() as abx:
                        projpsum = abx.enter_context(
                            tc.tile_pool(name="projpsum", bufs=1,
                                         space="PSUM"))
                        rpsum = abx.enter_context(
                            tc.tile_pool(name="rpsum", bufs=2, space="PSUM"))
                        pscE_pool = abx.enter_context(
                            tc.tile_pool(name="pscE", bufs=2, space="PSUM"))
                        rope = abx.enter_context(
                            tc.tile_pool(name="rope", bufs=2))

                        def proj_tiles(width):
                            return [projpsum.tile([128, width], F32,
                                                  tag=f"pj{m}",
                                                  name=f"pj{m}",
                                                  padded_shape=[128, 512])
                                    for m in range(NPAIR)]

                        def rope_stage1(pj_t, width, n_act):
                            """Evacuate projection PSUM to bf16 (frees the
                            banks for the next block).  First n_act pairs
                            go through ACT, the rest through DVE."""
                            xs_l = []
                            for m in range(NPAIR):
                                xs = rope.tile([128, width], B16,
                                               tag=f"xs{m}", name=f"xs{m}")
                                if m < n_act:
                                    nc.scalar.copy(xs[:], pj_t[m][:])
                                else:
                                    nc.vector.tensor_copy(xs[:], pj_t[m][:])
                                xs_l.append(xs)
                            return xs_l

                        def rope_tsins(pend):
                            xs_l, cos_ap, sin_ap, outs, width = pend
                            ts_l = []
                            for m in range(NPAIR):
                                tsin = rope.tile([128, width], B16,
                                                 tag="tsin", name="tsin",
                                                 bufs=4)
                                nc.vector.tensor_mul(tsin[:], xs_l[m][:],
                                                     sin_ap)
                                ts_l.append(tsin)
                            return ts_l

                        def rope_rot(pend, ts_l, m):
                            """Stage 2 for pair m: rotate matmul + cos mul
                            + add into qr/kr."""
                            xs_l, cos_ap, sin_ap, outs, width = pend
                            prot = rpsum.tile([128, width], F32, tag="prot",
                                              name="prot",
                                              padded_shape=[128, 512])
                            nc.tensor.matmul(prot[:], rt_t[:], ts_l[m][:],
                                             start=True, stop=True)
                            tcos = rope.tile([128, width], B16, tag="tcos",
                                             name="tcos")
                            nc.vector.tensor_mul(tcos[:], xs_l[m][:], cos_ap)
                            nc.vector.tensor_add(outs[m], tcos[:], prot[:])

                        def scores_exp(h, sc, psc_pool, tag="psc"):
                            """Scores for one key-chunk, exp into ptE."""
                            p, r0 = head_slices(h)
                            psc = psc_pool.tile([128, 512], F32,
                                                tag=tag, name="psc")
                            nc.tensor.matmul(
                                psc[:],
                                kr_t[p][r0:r0 + 64,
                                        sc * 128:(sc + 1) * 128],
                                qr_t[p][r0:r0 + 64, :],
                                start=True, stop=True)
                            nc.scalar.activation(
                                ptE[h][:, sc * 512:(sc + 1) * 512], psc[:],
                                EXP, scale=SCALE)

                        # ---- A: Q projection, per-slab streamed ----
                        with ExitStack() as ax:
                            apool = ax.enter_context(
                                tc.tile_pool(name="apool", bufs=4))
                            pq_t = proj_tiles(Nq)
                            for ci in range(8):
                                wq_c = apool.tile([128, CH], B16, tag="wqc",
                                                  name="wqc")
                                nc.sync.dma_start(
                                    wq_c[:], wqT[ci * 128:(ci + 1) * 128, :])
                                qt_c = apool.tile([128, Nq], B16, tag="qtc",
                                                  name="qtc")
                                nc.sync.dma_start(
                                    qt_c[:], qT[ci * 128:(ci + 1) * 128, :])
                                for m in range(NPAIR):
                                    nc.tensor.matmul(
                                        pq_t[m][:],
                                        wq_c[:, m * 128:(m + 1) * 128],
                                        qt_c[:],
                                        start=(ci == 0), stop=(ci == 7))
                            qc_t = qtbl.tile([128, Nq], B16, tag="qcos",
                                             name="qcos")
                            nc.sync.dma_start(qc_t[:], qcos[:])
                            qs_t = qtbl.tile([128, Nq], B16, tag="qsin",
                                             name="qsin")
                            nc.sync.dma_start(qs_t[:], qsin[:])
                            if not const_loaded:
                                const_loaded = True
                                nc.sync.dma_start(rt_t[:], rT[:])
                                nc.sync.dma_start(id_t[:], idT[:])
                                nc.sync.dma_start(bp_t[:], bpT[:])
                            xs_a = rope_stage1(pq_t, Nq, 2)
                            pending = (xs_a, qc_t[:], qs_t[:],
                                       [qr_t[m][:] for m in range(NPAIR)],
                                       Nq)

                        # ---- B: K projection blocks, deep-pipelined ----
                        wk_t = wstage.tile([128, 8 * CH], B16, tag="w",
                                           name="wk")
                        kt0_t = stream.tile([128, 4096], B16, tag="s",
                                            name="kt0")
                        for ci in range(8):
                            nc.sync.dma_start(
                                wk_t[:, ci * CH:(ci + 1) * CH],
                                wkT[ci * 128:(ci + 1) * 128, :])
                            nc.sync.dma_start(
                                kt0_t[:, ci * 512:(ci + 1) * 512],
                                kT[ci * 128:(ci + 1) * 128, 0:512])

                        score_q = []      # deferred (h, sc) score/exp work
                        for sbi in range(SB):
                            sl = slice(sbi * 512, (sbi + 1) * 512)
                            kc_t = ktbl.tile([128, 512], B16, tag="kcos",
                                             name="kcos")
                            nc.sync.dma_start(kc_t[:], kcos[:, sl])
                            ks_t = ktbl.tile([128, 512], B16, tag="ksin",
                                             name="ksin")
                            nc.sync.dma_start(ks_t[:], ksin[:, sl])
                            if sbi == 0:
                                kt_t = kt0_t
                            elif sbi == 1:
                                # block 0 left no DMA lookahead: stream
                                # block 1 per-slab too
                                kt_t = stream.tile([128, 4096], B16,
                                                   tag="s", name="kt")
                                for ci in range(8):
                                    nc.sync.dma_start(
                                        kt_t[:, ci * 512:(ci + 1) * 512],
                                        kT[ci * 128:(ci + 1) * 128, sl])
                            else:
                                kt_t = stream.tile([128, 4096], B16,
                                                   tag="s", name="kt")
                                _ld3(nc, kt_t, kT[:, sl])
                            pk_t = proj_tiles(512)
                            ts_l = rope_tsins(pending)
                            for ci in range(8):
                                for m in range(NPAIR):
                                    nc.tensor.matmul(
                                        pk_t[m][:],
                                        wk_t[:, ci * CH + m * 128:
                                             ci * CH + (m + 1) * 128],
                                        kt_t[:, ci * 512:(ci + 1) * 512],
                                        start=(ci == 0), stop=(ci == 7))
                                if 1 <= ci <= 4:
                                    rope_rot(pending, ts_l, ci - 1)
                                if ci >= 4:
                                    for _ in range(3):
                                        if score_q:
                                            h, sc = score_q.pop(0)
                                            scores_exp(h, sc, pscE_pool)
                            # this block's rope stage 1 + bookkeeping
                            xs_b = rope_stage1(pk_t, 512, 0)
                            pending = (xs_b, kc_t[:], ks_t[:],
                                       [kr_t[m][:, sl]
                                        for m in range(NPAIR)], 512)
                            for h in range(NB_EARLY):
                                for scj in range(4):
                                    score_q.append((h, sbi * 4 + scj))
                        # final block's rotate/add + leftover scores
                        ts_l = rope_tsins(pending)
                        for m in range(NPAIR):
                            rope_rot(pending, ts_l, m)
                        while len(score_q) > NB_EARLY * 4:
                            h, sc = score_q.pop(0)
                            scores_exp(h, sc, pscE_pool)
                        trail_scores = score_q   # last block: into pass0a

                # ===== pass0 + tail =====
                with ExitStack() as p0t:
                    ptL_pool = p0t.enter_context(tc.tile_pool(name="ptL",
                                                              bufs=3))
                    xpool = p0t.enter_context(tc.tile_pool(name="xpool",
                                                           bufs=1))
                    xq_t = [xpool.tile([128, CH], B16, tag=f"xq{qc}",
                                       name=f"xq{qc}") for qc in range(4)]
                    xt_t = [xpool.tile([128, Nq], B16, tag=f"xt{m}",
                                       name=f"xt{m}") for m in range(NPAIR)]
                    wp_t = xpool.tile([128, NPAIR * C], B16, tag="wp",
                                      name="wp")
                    nc.sync.dma_start(wp_t[:], wpT[:])
                    inv_pool = p0t.enter_context(tc.tile_pool(name="invp",
                                                              bufs=4))

                    # ones columns of v65 (col 64 of each 65-block)
                    nc.vector.memset(
                        v65[:].rearrange("p (n w) -> p n w", w=65)[:, :,
                                                                   64:65],
                        1.0)

                    ptL_t = {}

                    def late_group(h, gi, psc_pool):
                        """One 2-chunk scores+exp group for a mid/late
                        head, saved into its rotating prob buffer."""
                        if h not in ptL_t:
                            ptL_t[h] = ptL_pool.tile([128, SC * 512], B16,
                                                     tag="pt", name="pt")
                        ptl = ptL_t[h]
                        p, r0 = head_slices(h)
                        sc0 = gi * 2
                        psc = psc_pool.tile([128, 1024], F32, tag="psc",
                                            name="psc")
                        for j in range(2):
                            sc = sc0 + j
                            nc.tensor.matmul(
                                psc[:, j * 512:(j + 1) * 512],
                                kr_t[p][r0:r0 + 64,
                                        sc * 128:(sc + 1) * 128],
                                qr_t[p][r0:r0 + 64, :],
                                start=True, stop=True)
                        nc.scalar.activation(
                            ptl[:, sc0 * 512:(sc0 + 2) * 512], psc[:],
                            EXP, scale=SCALE)

                    # ---- pass0a: V projection + mid-head scores/exp ----
                    with ExitStack() as ph:
                        stream0 = ph.enter_context(
                            tc.tile_pool(name="stream0", bufs=2))
                        wstage0 = ph.enter_context(
                            tc.tile_pool(name="wstage0", bufs=1))
                        pv_pool = ph.enter_context(
                            tc.tile_pool(name="pv0", bufs=2, space="PSUM"))
                        pscM_pool = ph.enter_context(
                            tc.tile_pool(name="pscM", bufs=2, space="PSUM"))

                        def trail_score(h, sc):
                            """Leftover early-head score from phase B."""
                            p, r0 = head_slices(h)
                            psc = pscM_pool.tile([128, 512], F32,
                                                 tag="psc1", name="psc1")
                            nc.tensor.matmul(
                                psc[:],
                                kr_t[p][r0:r0 + 64,
                                        sc * 128:(sc + 1) * 128],
                                qr_t[p][r0:r0 + 64, :],
                                start=True, stop=True)
                            nc.scalar.activation(
                                ptE[h][:, sc * 512:(sc + 1) * 512], psc[:],
                                EXP, scale=SCALE)

                        mid_q = [(h, gi)
                                 for h in range(NB_EARLY, NB_EARLY + NB_MID)
                                 for gi in range(8)]
                        wv_t = wstage0.tile([128, 8 * CH], B16, tag="w",
                                            name="wv")
                        _ld3(nc, wv_t, wvT[:], CH)
                        for sbi in range(SB):
                            sl = slice(sbi * 512, (sbi + 1) * 512)
                            vt_t = stream0.tile([128, 4096], B16, tag="s",
                                                name="vt")
                            _ld3(nc, vt_t, vT[:, sl])
                            for scj in range(4):
                                sc = sbi * 4 + scj
                                pv = pv_pool.tile([128, CH], F32, tag="pv",
                                                  name="pv")
                                for ci in range(8):
                                    nc.tensor.matmul(
                                        pv[:],
                                        vt_t[:, ci * 512 + scj * 128:
                                             ci * 512 + (scj + 1) * 128],
                                        wv_t[:, ci * CH:(ci + 1) * CH],
                                        start=(ci == 0), stop=(ci == 7))
                                dst = v65[:, sc * 520:(sc + 1) * 520
                                          ].rearrange(
                                              "p (n w) -> p n w",
                                              w=65)[:, :, 0:64]
                                nc.vector.tensor_copy(
                                    dst,
                                    pv[:].rearrange("p (n w) -> p n w",
                                                    w=64))
                                if trail_scores:
                                    trail_score(*trail_scores.pop(0))
                                for _ in range(2):
                                    if mid_q:
                                        late_group(*mid_q.pop(0),
                                                   pscM_pool)

                    # ---- pass0b: transposed PV, one bank per group;
                    # heads 6/7 run scores+exp between PV groups ----
                    with ExitStack() as ph:
                        pvt_pool = ph.enter_context(
                            tc.tile_pool(name="pvt", bufs=4, space="PSUM"))
                        pscL_pool = ph.enter_context(
                            tc.tile_pool(name="pscL", bufs=2, space="PSUM"))

                        def pvt_head(h, pt_tile):
                            for qc in range(4):
                                acc = pvt_pool.tile(
                                    [128, 65], F32, tag="acc", name="acc",
                                    padded_shape=[128, 512])
                                for sc in range(SC):
                                    nc.tensor.matmul(
                                        acc[:],
                                        pt_tile[:, sc * 512 + qc * 128:
                                                sc * 512 + (qc + 1) * 128],
                                        v65[:, sc * 520 + h * 65:
                                            sc * 520 + (h + 1) * 65],
                                        start=(sc == 0), stop=(sc == SC - 1))
                                inv = inv_pool.tile([128, 1], F32, tag="inv",
                                                    name="inv")
                                nc.vector.reciprocal(inv[:], acc[:, 64:65])
                                nc.vector.tensor_scalar_mul(
                                    xq_t[qc][:, h * 64:(h + 1) * 64],
                                    acc[:, 0:64], inv[:])

                        late_q = [(6, gi) for gi in range(8)] + \
                                 [(7, gi) for gi in range(8)]
                        pvt_head(NB_EARLY, ptL_t[NB_EARLY])
                        plan = [(0, 3), (1, 3), (2, 2), (4, 3), (5, 3),
                                (6, 2), (7, 0)]
                        for h, ng in plan:
                            src_t = ptE[h] if h < NB_EARLY else ptL_t[h]
                            pvt_head(h, src_t)
                            for _ in range(ng):
                                if late_q:
                                    late_group(*late_q.pop(0), pscL_pool)

                    # ---- tail: transpose -> out proj ----
                    with ExitStack() as tl:
                        tp_pool = tl.enter_context(
                            tc.tile_pool(name="tp", bufs=4, space="PSUM"))
                        po_pool = tl.enter_context(
                            tc.tile_pool(name="po", bufs=4, space="PSUM"))
                        osb_pool = tl.enter_context(
                            tc.tile_pool(name="osb", bufs=3))
                        for m in range(NPAIR):
                            for qc in range(4):
                                tp = tp_pool.tile([128, 128], F32, tag="tp",
                                                  name="tp",
                                                  padded_shape=[128, 512])
                                nc.tensor.matmul(
                                    tp[:],
                                    xq_t[qc][:, m * 128:(m + 1) * 128],
                                    id_t[:], start=True, stop=True)
                                if m < 2:
                                    nc.scalar.copy(
                                        xt_t[m][:, qc * 128:(qc + 1) * 128],
                                        tp[:])
                                else:
                                    nc.vector.tensor_copy(
                                        xt_t[m][:, qc * 128:(qc + 1) * 128],
                                        tp[:])
                        for j in range(8):
                            po = po_pool.tile([128, Nq], F32, tag="po",
                                              name="po")
                            for m in range(NPAIR):
                                nc.tensor.matmul(
                                    po[:],
                                    wp_t[:, m * C + j * 128:
                                         m * C + (j + 1) * 128],
                                    xt_t[m][:], start=(m == 0),
                                    stop=(m == NPAIR - 1))
                            osb = osb_pool.tile([128, Nq], B16, tag="osb",
                                                name="osb")
                            nc.scalar.activation(
                                osb[:], po[:],
                                mybir.ActivationFunctionType.Identity,
                                bias=bp_t[:, j:j + 1])
                            nc.sync.dma_start(
                                outT[j * 128:(j + 1) * 128, :], osb[:])

    nc.compile()
    return nc


def prep_inputs(query, key, value, qpos, kpos, Wq, Wk, Wv, Wp, bp):
    """Build per-core input maps (8 cores: core = 2*b + g)."""
    bf16 = ml_dtypes.bfloat16
    invf = (1.0 / ROPE_BASE ** (np.arange(0, D, 2, dtype=np.float32) / D)
            ).astype(np.float32)
    rows64 = invf[np.arange(64) % 32]          # [64]

    R64 = np.zeros((64, 64), dtype=np.float32)
    for r in range(32):
        R64[r, r + 32] = -1.0
        R64[r + 32, r] = 1.0
    rT128 = np.zeros((128, 128), dtype=np.float32)
    rT128[0:64, 0:64] = R64.T
    rT128[64:128, 64:128] = R64.T

    def b(x):
        return np.ascontiguousarray(np.asarray(x, np.float32)).astype(bf16)

    in_maps = []
    for core in range(8):
        bi, g = core // 2, core % 2
        cols = slice(g * CH, (g + 1) * CH)
        qang = rows64[:, None] * np.asarray(qpos[bi], np.float32)[None, :]
        kang = rows64[:, None] * np.asarray(kpos[bi], np.float32)[None, :]
        Wg = np.asarray(Wp, np.float32)[:, cols]        # [C, CH]
        wp_pair = Wg.reshape(C, NPAIR, 128).transpose(2, 1, 0).reshape(
            128, NPAIR * C)
        m = {
            "qT": b(np.asarray(query[bi], np.float32).T),
            "kT": b(np.asarray(key[bi], np.float32).T),
            "vT": b(np.asarray(value[bi], np.float32).T),
            "wqT": b(np.asarray(Wq, np.float32)[cols, :].T),
            "wkT": b(np.asarray(Wk, np.float32)[cols, :].T),
            "wvT": b(np.asarray(Wv, np.float32)[cols, :].T),
            "wpT": b(wp_pair),
            "bpT": (np.ascontiguousarray(
                        np.asarray(bp, np.float32).reshape(8, 128).T)
                    if g == 0 else np.zeros((128, 8), np.float32)),
            "qcos": b(np.tile(np.cos(qang), (2, 1))),
            "qsin": b(np.tile(np.sin(qang), (2, 1))),
            "kcos": b(np.tile(np.cos(kang), (2, 1))),
            "ksin": b(np.tile(np.sin(kang), (2, 1))),
            "rT": b(rT128),
            "idT": b(np.eye(128, dtype=np.float32)),
        }
        in_maps.append(m)
    return in_maps


_NC_CACHE = {}


def _get_nc(iters=1):
    if iters not in _NC_CACHE:
        _NC_CACHE[iters] = build_nc(iters)
    return _NC_CACHE[iters]


def kernel(query, key, value, qpos, kpos, Wq, Wk, Wv, Wp, bp):
    from concourse.bass_utils import run_bass_kernel_spmd

    nc = _get_nc()
    in_maps = prep_inputs(query, key, value, qpos, kpos, Wq, Wk, Wv, Wp, bp)
    res = run_bass_kernel_spmd(nc, in_maps, list(range(8)))
    out = np.zeros((B, Nq, C), dtype=np.float32)
    for core in range(8):
        out[core // 2] += np.asarray(res.results[core]["outT"],
                             dtype=np.float32).T
    return out
